# revision 4
# baseline (speedup 1.0000x reference)
"""CoAttention Trainium2 kernel.

Problem: B=16, PLEN=1024, QLEN=256, D=256 fp32.
  score[b,p,q] = passage.w_p + question.w_q + (passage*w_pq).question + b
  masked-softmax both ways, three attention matmuls.

Data-parallel over batch across 8 NeuronCores (2 batches/core), everything
per-batch local.

Math (per batch), with E0 = exp(S0), S0[p,q] = (P w_pq)·Q:
  ET'[q,p] = exp(S0[p,q] + sq[q] + b - 1e7*qm[q] - C)   (bias per q-partition)
  h[p]     = exp(sp[p]) * (1-pm[p])
  p2q[p,:] = (ET'^T @ [Q|1]) * kp/dp    (dp from the ones column)
  q2p[q,:] = (E0 @ [P*h|h]) / dq        (dq from the h column)
  coatt    = (ET'^T @ q2p) * kp/dp
The q-side softmax factor g rides inside ET' as an exp bias; the p-side
factors (exp(sp), mask) ride in h. The constant shift C cancels in the
normalizations and keeps ET' inside fp8e4 range.

Engine plan: scores (S0, ST) are fp32r matmuls (1 cyc/row at >=256 moving);
E0/ET' are written by ACT exp directly as fp8e4; the attention matmuls
(p2q, q2p, coatt) run fp8 DoubleRow (2 K-tiles per matmul, 0.5 cyc/row).
S0 tiles pair two p-tiles per PSUM bank so each exp is 512 wide.
PSUM->SBUF drains alternate ACT/DVE; fp8 conversions and small SBUF ops go
to Pool; inputs stream on the SP DMA queue, outputs on the Pool queue.

The container's walrus accepts only ONE sync-wait per non-matmul
instruction (and none on matmuls); a BIR post-pass splits waits into
single-wait EventSemaphore carriers. All matmul moving dims are even.
"""

import numpy as np
import orjson

import concourse.bass as bass
import concourse.mybir as mybir
import concourse.tile as tile
from concourse.bass_utils import run_bass_kernel_spmd
from concourse.masks import make_identity

F32 = mybir.dt.float32
F32R = mybir.dt.float32r
F8 = mybir.dt.float8e4
I32 = mybir.dt.int32
AF = mybir.ActivationFunctionType
DR = mybir.MatmulPerfMode.DoubleRow

N_CORES = 8
B, PLEN, QLEN, D = 16, 1024, 256, 256
NB = B // N_CORES  # batches per core
PT_T = PLEN // 128  # 8 p-tiles
QT_T = QLEN // 128  # 2 q-tiles
DT_T = D // 128  # 2 d-tiles
MASK = -10000000.0
CSH = 1.5  # ET' exp shift; cancels in softmax, keeps fp8 in range

# ---------------------------------------------------------------------------
# walrus single-wait workaround


def _split_waits_in_bir(bir: dict) -> None:
    for f in bir.get("functions", []):
        for blk in f.get("blocks", []):
            out = []
            for i in blk.get("instructions", []):
                si = i.get("sync_info")
                ow = (si or {}).get("on_wait") or []
                limit = 0 if i.get("opcode") == "Matmult" else 1
                if len(ow) > limit:
                    for k, w in enumerate(ow[limit:]):
                        out.append(
                            {
                                "debug": i.get("debug"),
                                "engine": i["engine"],
                                "ins": [],
                                "outs": [],
                                "name": f"{i['name']}__w{k}",
                                "opcode": "EventSemaphore",
                                "sync_info": {"on_update": [], "on_wait": [w]},
                            }
                        )
                    si["on_wait"] = ow[:limit]
                out.append(i)
            blk["instructions"] = out


_patched = False


def _install_bir_wait_split():
    global _patched
    if _patched:
        return
    _patched = True
    import concourse.bass2jax as b2j
    import concourse.bass_utils as bu

    orig = bu.compile_bir_kernel

    def patched(bir_json, tmpdir, neff_name="file.neff"):
        bir = orjson.loads(bir_json)
        _split_waits_in_bir(bir)
        return orig(orjson.dumps(bir), tmpdir, neff_name)

    bu.compile_bir_kernel = patched
    b2j.compile_bir_kernel = patched


# ---------------------------------------------------------------------------


def build_nc(bufs_cfg=None) -> bass.Bass:
    cfg = {"tp": 2, "s0": 2, "st": 1, "at": 2, "mi": 1, "big": 2, "small": 2}
    if bufs_cfg:
        cfg.update(bufs_cfg)
    nc = bass.Bass()
    passage = nc.declare_dram_parameter("passage", [NB, PLEN, D], F32, isOutput=False)
    question = nc.declare_dram_parameter("question", [NB, QLEN, D], F32, isOutput=False)
    pmask = nc.declare_dram_parameter("passage_mask", [NB, PLEN], I32, isOutput=False)
    qmask = nc.declare_dram_parameter("question_mask", [NB, QLEN], I32, isOutput=False)
    w_all = nc.declare_dram_parameter("W", [3 * D], F32, isOutput=False)
    b_in = nc.declare_dram_parameter("b", [1], F32, isOutput=False)
    out_p2q = nc.declare_dram_parameter("p2q", [NB, PLEN, D], F32, isOutput=True)
    out_co = nc.declare_dram_parameter("coatt", [NB, PLEN, D], F32, isOutput=True)

    with tile.TileContext(nc) as tc:
        with (
            tc.tile_pool(name="const", bufs=1) as const_pool,
            tc.tile_pool(name="big", bufs=cfg["big"]) as big,
            tc.tile_pool(name="small", bufs=cfg["small"]) as small,
            tc.tile_pool(name="tp_ps", bufs=cfg["tp"], space="PSUM") as tp_ps,
            tc.tile_pool(name="s0_ps", bufs=cfg["s0"], space="PSUM") as s0_ps,
            tc.tile_pool(name="st_ps", bufs=cfg["st"], space="PSUM") as st_ps,
            tc.tile_pool(name="at_ps", bufs=cfg["at"], space="PSUM") as at_ps,
            tc.tile_pool(name="mi_ps", bufs=cfg["mi"], space="PSUM") as mi_ps,
        ):
            # ---- ACT exp table warm-up (off the critical path) ----------
            warm_in = const_pool.tile([128, 2], F32, name="warm_in")
            nc.gpsimd.memset(warm_in[:], 0.0)
            warm_out = const_pool.tile([128, 2], F32, name="warm_out")
            nc.scalar.activation(warm_out[:], warm_in[:], AF.Exp)

            ident = const_pool.tile([128, 128], F32, name="ident")
            make_identity(nc, ident[:])
            ident_r_t = const_pool.tile([128, 128], F32R, name="ident_r_t")
            nc.vector.tensor_copy(ident_r_t[:], ident[:])
            ident_r = ident_r_t[:]

            onesf = const_pool.tile([128, 2], F32, name="onesf")
            nc.gpsimd.memset(onesf[:], 1.0)
            ones8 = const_pool.tile([128, 2], F8, name="ones8")
            nc.vector.tensor_copy(ones8[:], onesf[:])

            # weight columns: [d_in_tile, k]  cols: wp0 wp1 wq0 wq1 wpq0 wpq1
            w6 = const_pool.tile([128, 6], F32, name="w6")
            nc.gpsimd.dma_start(w6[:], w_all[:].rearrange("(k d) -> d k", d=128))
            w_pq = w6[:, 2 * DT_T : 3 * DT_T]
            # duplicated 2-wide weight columns for the tiny sp/sq matmuls
            w_p_r = const_pool.tile([128, DT_T, 2], F32R, name="w_p_r")
            w_q_r = const_pool.tile([128, DT_T, 2], F32R, name="w_q_r")
            for j in range(DT_T):
                for k in range(2):
                    nc.vector.tensor_copy(w_p_r[:, j, k : k + 1], w6[:, j : j + 1])
                    nc.vector.tensor_copy(
                        w_q_r[:, j, k : k + 1], w6[:, DT_T + j : DT_T + j + 1]
                    )

            # masks for all batches + bias
            pm_all = const_pool.tile([128, NB, PT_T], I32, name="pm_all")
            nc.gpsimd.dma_start(pm_all[:], pmask[:].rearrange("n (t p) -> p n t", p=128))
            qm_all = const_pool.tile([128, NB, QT_T], I32, name="qm_all")
            nc.gpsimd.dma_start(qm_all[:], qmask[:].rearrange("n (t q) -> q n t", q=128))
            b_sb = const_pool.tile([128, 1], F32, name="b_sb")
            nc.gpsimd.dma_start(b_sb[:], b_in[0:1].partition_broadcast(128))
            bm1 = const_pool.tile([128, 1], F32, name="bm1")
            nc.vector.tensor_scalar_add(bm1[:], b_sb[:], -CSH)

            def emit_batch(bi):
                p2q_dst = out_p2q[bi].rearrange("(t p) d -> p t d", p=128)
                co_dst = out_co[bi].rearrange("(t p) d -> p t d", p=128)

                # ---- phase: input DMAs (SP queue) -----------------------
                q_sb = small.tile([128, QT_T, D], F32R, name="q_sb", tag="q_sb")
                q_src = question[bi].rearrange("(t q) d -> q t d", q=128).bitcast(F32R)
                p_sb = big.tile([128, PT_T, D], F32R, name="p_sb", tag="p_sb")
                p_src = passage[bi].rearrange("(t p) d -> p t d", p=128).bitcast(F32R)
                nc.sync.dma_start(q_sb[:, 0:1, :], q_src[:, 0:1, :])
                nc.sync.dma_start(p_sb[:, 0:2, :], p_src[:, 0:2, :])
                nc.sync.dma_start(q_sb[:, 1:2, :], q_src[:, 1:2, :])
                for c in range(1, 4):
                    nc.sync.dma_start(
                        p_sb[:, 2 * c : 2 * c + 2, :], p_src[:, 2 * c : 2 * c + 2, :]
                    )
                yield "dma"

                # ---- phase: head (masks, Q transposes, sq, gb, q8) ------
                pm_f = small.tile([128, PT_T], F32, name="pm_f", tag="pm_f")
                nc.vector.tensor_copy(pm_f[:], pm_all[:, bi])
                kp = small.tile([128, PT_T], F32, name="kp", tag="kp")
                nc.vector.tensor_scalar(
                    kp[:], pm_f[:], -1.0, 1.0, mybir.AluOpType.mult, mybir.AluOpType.add
                )
                qm_f = small.tile([128, QT_T], F32, name="qm_f", tag="qm_f")
                nc.vector.tensor_copy(qm_f[:], qm_all[:, bi])
                # qmb = qm*MASK + (b - C)
                qmb = small.tile([128, QT_T], F32, name="qmb", tag="qmb")
                nc.vector.tensor_scalar(
                    qmb[:],
                    qm_f[:],
                    MASK,
                    bm1[:, 0:1],
                    mybir.AluOpType.mult,
                    mybir.AluOpType.add,
                )

                # Q transposes: per-j bank holds both q-tiles side by side
                qt_r = small.tile([128, DT_T, QLEN], F32R, name="qt_r", tag="qt_r")
                qwt = small.tile([128, DT_T, QLEN], F32R, name="qwt", tag="qwt")
                for j in range(DT_T):
                    tq = tp_ps.tile([128, 256], F32R, name=f"tq{j}", tag="tp")
                    for t4 in range(QT_T):
                        nc.tensor.transpose(
                            tq[:, t4 * 128 : (t4 + 1) * 128],
                            q_sb[:, t4, j * 128 : (j + 1) * 128],
                            ident_r,
                        )
                    # QwT = QT * w_pq (per-partition d scale)
                    nc.vector.tensor_scalar_mul(qwt[:, j, :], tq[:], w_pq[:, j : j + 1])
                    nc.scalar.copy(qt_r[:, j, :], tq[:])

                # q8 = [Q | 1 | 1] in fp8, [q_part, tq, 258]
                q8 = small.tile([128, QT_T, QLEN + 2], F8, name="q8", tag="q8")
                for t4 in range(QT_T):
                    nc.gpsimd.tensor_copy(
                        q8[:, t4, 0:QLEN], q_sb[:, t4, :].bitcast(F32)
                    )
                    nc.gpsimd.tensor_copy(q8[:, t4, QLEN : QLEN + 2], ones8[:])

                # misc PSUM bank: cols 0:4 = sq (dup pairs), 4:20 = sp
                mi = mi_ps.tile([128, 4 + 2 * PT_T], F32, name="mi", tag="mi")
                sq = mi[:, 0 : 2 * QT_T]
                for tq_i in range(QT_T):
                    for j in range(DT_T):
                        nc.tensor.matmul(
                            sq[:, 2 * tq_i : 2 * tq_i + 2],
                            qt_r[:, j, tq_i * 128 : (tq_i + 1) * 128],
                            w_q_r[:, j, :],
                            start=(j == 0),
                            stop=(j == DT_T - 1),
                        )
                gb = small.tile([128, QT_T], F32, name="gb", tag="gb")
                for tq_i in range(QT_T):
                    nc.vector.tensor_add(
                        gb[:, tq_i : tq_i + 1],
                        sq[:, 2 * tq_i : 2 * tq_i + 1],
                        qmb[:, tq_i : tq_i + 1],
                    )
                yield "head"

                # ---- per-batch big tiles --------------------------------
                pt_r = big.tile([128, DT_T, PLEN], F32R, name="pt_r", tag="pt_r")
                # E0 pairs: [p_part, pair, 512] fp8 (tile 2u in cols 0:256)
                e_sb = big.tile([128, PT_T // 2, 512], F8, name="e_sb", tag="e_sb")
                et_sb = big.tile([128, QT_T, PLEN], F8, name="et_sb", tag="et_sb")
                ph8 = big.tile([128, PT_T, D + 2], F8, name="ph8", tag="ph8")
                q2p8 = small.tile([128, QT_T, D], F8, name="q2p8", tag="q2p8")
                p2q_sb = big.tile([128, PT_T, D], F32, name="p2q_sb", tag="p2q_sb")
                co_sb = big.tile([128, PT_T, D], F32, name="co_sb", tag="co_sb")
                rp = small.tile([128, PT_T], F32, name="rp", tag="rp")
                sp = mi[:, 2 * QT_T : 2 * QT_T + 2 * PT_T]

                def emit_ap(t):
                    # p2q attention for tile t: one fp8 DoubleRow matmul
                    ap_ = at_ps.tile([128, QLEN + 2], F32, name="ap_", tag="at")
                    nc.tensor.matmul(
                        ap_[:],
                        et_sb[:, :, t * 128 : (t + 1) * 128],
                        q8[:],
                        start=True,
                        stop=True,
                        perf_mode=DR,
                    )
                    v2 = small.tile([128, 1], F32, name="v2", tag="v2")
                    nc.vector.reciprocal(v2[:], ap_[:, QLEN : QLEN + 1])
                    nc.vector.tensor_mul(rp[:, t : t + 1], v2[:], kp[:, t : t + 1])
                    if t % 2 == 0:
                        nc.scalar.activation(
                            p2q_sb[:, t, :], ap_[:, 0:QLEN], AF.Copy,
                            scale=rp[:, t : t + 1],
                        )
                    else:
                        nc.vector.tensor_scalar_mul(
                            p2q_sb[:, t, :], ap_[:, 0:QLEN], rp[:, t : t + 1]
                        )
                    nc.sync.dma_start(p2q_dst[:, t : t + 1, :], p2q_sb[:, t : t + 1, :])

                def emit_co(t):
                    co = at_ps.tile([128, D], F32, name="co", tag="at")
                    nc.tensor.matmul(
                        co[:],
                        et_sb[:, :, t * 128 : (t + 1) * 128],
                        q2p8[:],
                        start=True,
                        stop=True,
                        perf_mode=DR,
                    )
                    if t % 2 == 1:
                        nc.scalar.activation(
                            co_sb[:, t, :], co[:], AF.Copy, scale=rp[:, t : t + 1]
                        )
                    else:
                        nc.vector.tensor_scalar_mul(
                            co_sb[:, t, :], co[:], rp[:, t : t + 1]
                        )
                    nc.gpsimd.dma_start(co_dst[:, t : t + 1, :], co_sb[:, t : t + 1, :])

                def emit_grp_scores(grp):
                    t_lo = grp * 4
                    # PT transposes for this half (per-j bank of 4 tiles)
                    for j in range(DT_T):
                        tp = tp_ps.tile([128, 512], F32R, name="tp", tag="tp")
                        for t4 in range(4):
                            t = t_lo + t4
                            nc.tensor.transpose(
                                tp[:, t4 * 128 : (t4 + 1) * 128],
                                p_sb[:, t, j * 128 : (j + 1) * 128],
                                ident_r,
                            )
                        if j == 0:
                            nc.vector.tensor_copy(
                                pt_r[:, j, grp * 512 : (grp + 1) * 512], tp[:]
                            )
                        else:
                            nc.scalar.copy(
                                pt_r[:, j, grp * 512 : (grp + 1) * 512], tp[:]
                            )
                    # S0 pairs -> paired exp -> fp8 E0; sp rides along
                    for pair in range(2):
                        t0 = t_lo + 2 * pair
                        s0b = s0_ps.tile([128, 512], F32, name="s0b", tag="s0")
                        for half in range(2):
                            t = t0 + half
                            for j in range(DT_T):
                                nc.tensor.matmul(
                                    s0b[:, half * 256 : (half + 1) * 256],
                                    pt_r[:, j, t * 128 : (t + 1) * 128],
                                    qwt[:, j, :],
                                    start=(j == 0),
                                    stop=(j == DT_T - 1),
                                )
                            for j in range(DT_T):
                                nc.tensor.matmul(
                                    sp[:, 2 * t : 2 * t + 2],
                                    pt_r[:, j, t * 128 : (t + 1) * 128],
                                    w_p_r[:, j, :],
                                    start=(j == 0),
                                    stop=(j == DT_T - 1),
                                )
                        nc.scalar.activation(e_sb[:, t0 // 2, :], s0b[:], AF.Exp)
                    # ST pairs -> exp with gb bias -> fp8 ET'
                    for tq_i in range(QT_T):
                        stb = st_ps.tile([128, 512], F32, name="stb", tag="st")
                        for j in range(DT_T):
                            nc.tensor.matmul(
                                stb[:],
                                qwt[:, j, tq_i * 128 : (tq_i + 1) * 128],
                                pt_r[:, j, grp * 512 : (grp + 1) * 512],
                                start=(j == 0),
                                stop=(j == DT_T - 1),
                            )
                        nc.scalar.activation(
                            et_sb[:, tq_i, grp * 512 : (grp + 1) * 512],
                            stb[:],
                            AF.Exp,
                            bias=gb[:, tq_i : tq_i + 1],
                        )

                # ---- phase g0: first half scores + its p2q --------------
                emit_grp_scores(0)
                for t in range(0, 4):
                    emit_ap(t)
                yield "g0"

                # ---- phase g1a: second half scores + h/ph8 --------------
                emit_grp_scores(1)
                # h = exp(sp) * kp ; ph8 = [P*h | h | h] fp8
                h_raw = small.tile([128, 2 * PT_T], F32, name="h_raw", tag="h_raw")
                nc.scalar.activation(h_raw[:], sp[:], AF.Exp)
                hk = small.tile([128, 2 * PT_T], F32, name="hk", tag="hk")
                for t in range(PT_T):
                    nc.gpsimd.tensor_mul(
                        hk[:, 2 * t : 2 * t + 1],
                        h_raw[:, 2 * t : 2 * t + 1],
                        kp[:, t : t + 1],
                    )
                for t in range(PT_T):
                    nc.gpsimd.tensor_scalar_mul(
                        ph8[:, t, 0:D],
                        p_sb[:, t, :].bitcast(F32),
                        hk[:, 2 * t : 2 * t + 1],
                    )
                    nc.gpsimd.tensor_copy(
                        ph8[:, t, D : D + 1], hk[:, 2 * t : 2 * t + 1]
                    )
                    nc.gpsimd.tensor_copy(
                        ph8[:, t, D + 1 : D + 2], hk[:, 2 * t : 2 * t + 1]
                    )
                yield "g1a"

                # ---- phase g1b: q2p, remaining p2q, coattention ---------
                for tq_i in range(QT_T):
                    aq = at_ps.tile([128, D + 2], F32, name="aq", tag="at")
                    for u in range(PT_T // 2):
                        nc.tensor.matmul(
                            aq[:],
                            e_sb[:, u, :]
                            .rearrange("p (k q) -> p k q", k=2)[
                                :, :, tq_i * 128 : (tq_i + 1) * 128
                            ],
                            ph8[:, 2 * u : 2 * u + 2, :],
                            start=(u == 0),
                            stop=(u == PT_T // 2 - 1),
                            perf_mode=DR,
                        )
                    u2 = small.tile([128, 1], F32, name="u2", tag="u2")
                    nc.vector.reciprocal(u2[:], aq[:, D : D + 1])
                    nc.vector.tensor_scalar_mul(q2p8[:, tq_i, :], aq[:, 0:D], u2[:])
                for t in range(4, PT_T):
                    emit_ap(t)
                    emit_co(t - 4)
                for t in range(4, PT_T):
                    emit_co(t)
                yield "g1b"

            # interleaved emission: batch 1's input DMAs and head overlap
            # batch 0's compute; see docstring.
            gens = [emit_batch(bi) for bi in range(NB)]
            if NB == 2:
                g0, g1 = gens
                next(g0)  # b0 dma
                next(g0)  # b0 head
                next(g0)  # b0 g0
                next(g1)  # b1 dma
                next(g0)  # b0 g1a
                next(g1)  # b1 head
                next(g0)  # b0 g1b
                next(g1)  # b1 g0
                next(g1)  # b1 g1a
                next(g1)  # b1 g1b
                for g in gens:
                    for _ in g:
                        pass
            else:
                for g in gens:
                    for _ in g:
                        pass

    return nc


_nc_cache = None


def kernel(passage, question, passage_mask, question_mask, W, b):
    global _nc_cache
    _install_bir_wait_split()
    if _nc_cache is None:
        _nc_cache = build_nc()
    nc = _nc_cache

    passage = np.ascontiguousarray(passage, dtype=np.float32)
    question = np.ascontiguousarray(question, dtype=np.float32)
    passage_mask = np.ascontiguousarray(passage_mask, dtype=np.int32)
    question_mask = np.ascontiguousarray(question_mask, dtype=np.int32)
    W = np.ascontiguousarray(W, dtype=np.float32)
    b = np.ascontiguousarray(b, dtype=np.float32)

    in_maps = []
    for c in range(N_CORES):
        s = slice(c * NB, (c + 1) * NB)
        in_maps.append(
            {
                "passage": passage[s],
                "question": question[s],
                "passage_mask": passage_mask[s],
                "question_mask": question_mask[s],
                "W": W,
                "b": b,
            }
        )
    res = run_bass_kernel_spmd(nc, in_maps, list(range(N_CORES)))
    p2q = np.concatenate([r["p2q"] for r in res.results], axis=0)
    coatt = np.concatenate([r["coatt"] for r in res.results], axis=0)
    return p2q, coatt


# revision 5
# speedup vs baseline: 1.0128x; 1.0128x over previous
"""CoAttention Trainium2 kernel.

Problem: B=16, PLEN=1024, QLEN=256, D=256 fp32.
  score[b,p,q] = passage.w_p + question.w_q + (passage*w_pq).question + b
  masked-softmax both ways, three attention matmuls.

Data-parallel over batch across 8 NeuronCores (2 batches/core), everything
per-batch local.

Math (per batch), with E0 = exp(S0), S0[p,q] = (P w_pq)·Q:
  g[q]     = exp(sq[q] + b - 1e7*qm[q])
  ET'[q,p] = E0[p,q]^T * g[q]        (PE transpose of E0, g rides the drain)
  h[p]     = exp(sp[p]) * (1-pm[p])
  p2q[p,:] = (ET'^T @ [Q|1]) * kp/dp   (dp from the ones column)
  q2p[q,:] = (E0 @ [P*h|h]) / dq       (dq from the h column)
  coatt    = (ET'^T @ q2p) * kp/dp
Row-constant softmax factors cancel; masks enter as exact zeros in g/h.

Engine plan: scores are fp32r matmuls over bf16-rounded operands (1
cyc/row at >=256 moving); E0 is written by ACT exp directly as bf16; ET'
comes from PE transposes of E0 (bf16, 1 cyc/row) instead of a second
score matmul + exp — saving both PE cycles and ACT exp work. Attention
matmuls run bf16. S0 tiles pair two p-tiles per PSUM bank so each exp is
512 wide. PSUM->SBUF drains alternate ACT/DVE; bf16 conversions and
small SBUF ops go to Pool; inputs stream on the SP DMA queue, outputs on
SP (p2q) and Pool (coatt) queues, one tile per DMA.

The container's walrus accepts only ONE sync-wait per non-matmul
instruction (and none on matmuls); a BIR post-pass splits waits into
single-wait EventSemaphore carriers. All matmul moving dims are even.
"""

import numpy as np
import orjson

import concourse.bass as bass
import concourse.mybir as mybir
import concourse.tile as tile
from concourse.bass_utils import run_bass_kernel_spmd
from concourse.masks import make_identity

F32 = mybir.dt.float32
F32R = mybir.dt.float32r
BF16 = mybir.dt.bfloat16
I32 = mybir.dt.int32
AF = mybir.ActivationFunctionType

N_CORES = 8
B, PLEN, QLEN, D = 16, 1024, 256, 256
NB = B // N_CORES  # batches per core
PT_T = PLEN // 128  # 8 p-tiles
QT_T = QLEN // 128  # 2 q-tiles
DT_T = D // 128  # 2 d-tiles
MASK = -10000000.0

# ---------------------------------------------------------------------------
# walrus single-wait workaround


def _split_waits_in_bir(bir: dict) -> None:
    for f in bir.get("functions", []):
        for blk in f.get("blocks", []):
            out = []
            for i in blk.get("instructions", []):
                si = i.get("sync_info")
                ow = (si or {}).get("on_wait") or []
                limit = 0 if i.get("opcode") == "Matmult" else 1
                if len(ow) > limit:
                    for k, w in enumerate(ow[limit:]):
                        out.append(
                            {
                                "debug": i.get("debug"),
                                "engine": i["engine"],
                                "ins": [],
                                "outs": [],
                                "name": f"{i['name']}__w{k}",
                                "opcode": "EventSemaphore",
                                "sync_info": {"on_update": [], "on_wait": [w]},
                            }
                        )
                    si["on_wait"] = ow[:limit]
                out.append(i)
            blk["instructions"] = out


_patched = False


def _install_bir_wait_split():
    global _patched
    if _patched:
        return
    _patched = True
    import concourse.bass2jax as b2j
    import concourse.bass_utils as bu

    orig = bu.compile_bir_kernel

    def patched(bir_json, tmpdir, neff_name="file.neff"):
        bir = orjson.loads(bir_json)
        _split_waits_in_bir(bir)
        return orig(orjson.dumps(bir), tmpdir, neff_name)

    bu.compile_bir_kernel = patched
    b2j.compile_bir_kernel = patched


# ---------------------------------------------------------------------------


def build_nc(bufs_cfg=None) -> bass.Bass:
    cfg = {"tp": 2, "s0": 2, "et": 1, "at": 2, "mi": 1, "big": 2, "small": 2}
    if bufs_cfg:
        cfg.update(bufs_cfg)
    nc = bass.Bass()
    passage = nc.declare_dram_parameter("passage", [NB, PLEN, D], F32, isOutput=False)
    question = nc.declare_dram_parameter("question", [NB, QLEN, D], F32, isOutput=False)
    pmask = nc.declare_dram_parameter("passage_mask", [NB, PLEN], I32, isOutput=False)
    qmask = nc.declare_dram_parameter("question_mask", [NB, QLEN], I32, isOutput=False)
    w_all = nc.declare_dram_parameter("W", [3 * D], F32, isOutput=False)
    b_in = nc.declare_dram_parameter("b", [1], F32, isOutput=False)
    out_p2q = nc.declare_dram_parameter("p2q", [NB, PLEN, D], F32, isOutput=True)
    out_co = nc.declare_dram_parameter("coatt", [NB, PLEN, D], F32, isOutput=True)

    with tile.TileContext(nc) as tc:
        with (
            tc.tile_pool(name="const", bufs=1) as const_pool,
            tc.tile_pool(name="big", bufs=cfg["big"]) as big,
            tc.tile_pool(name="small", bufs=cfg["small"]) as small,
            tc.tile_pool(name="tp_ps", bufs=cfg["tp"], space="PSUM") as tp_ps,
            tc.tile_pool(name="s0_ps", bufs=cfg["s0"], space="PSUM") as s0_ps,
            tc.tile_pool(name="et_ps", bufs=cfg["et"], space="PSUM") as et_ps,
            tc.tile_pool(name="at_ps", bufs=cfg["at"], space="PSUM") as at_ps,
            tc.tile_pool(name="mi_ps", bufs=cfg["mi"], space="PSUM") as mi_ps,
        ):
            # ---- ACT exp table warm-up (off the critical path) ----------
            warm_in = const_pool.tile([128, 2], F32, name="warm_in")
            nc.gpsimd.memset(warm_in[:], 0.0)
            warm_out = const_pool.tile([128, 2], F32, name="warm_out")
            nc.scalar.activation(warm_out[:], warm_in[:], AF.Exp)

            ident = const_pool.tile([128, 128], F32, name="ident")
            make_identity(nc, ident[:])
            ident_r_t = const_pool.tile([128, 128], F32R, name="ident_r_t")
            nc.vector.tensor_copy(ident_r_t[:], ident[:])
            ident_r = ident_r_t[:]
            ident_b_t = const_pool.tile([128, 128], BF16, name="ident_b_t")
            nc.gpsimd.tensor_copy(ident_b_t[:], ident[:])
            ident_b = ident_b_t[:]

            onesf = const_pool.tile([128, 2], F32, name="onesf")
            nc.gpsimd.memset(onesf[:], 1.0)
            ones_b = const_pool.tile([128, 2], BF16, name="ones_b")
            nc.vector.tensor_copy(ones_b[:], onesf[:])

            # weight columns: [d_in_tile, k]  cols: wp0 wp1 wq0 wq1 wpq0 wpq1
            w6 = const_pool.tile([128, 6], F32, name="w6")
            nc.gpsimd.dma_start(w6[:], w_all[:].rearrange("(k d) -> d k", d=128))
            w_pq = w6[:, 2 * DT_T : 3 * DT_T]
            # duplicated 2-wide weight columns for the tiny sp/sq matmuls
            w_p_r = const_pool.tile([128, DT_T, 2], BF16, name="w_p_r")
            w_q_r = const_pool.tile([128, DT_T, 2], BF16, name="w_q_r")
            for j in range(DT_T):
                for k in range(2):
                    nc.vector.tensor_copy(w_p_r[:, j, k : k + 1], w6[:, j : j + 1])
                    nc.vector.tensor_copy(
                        w_q_r[:, j, k : k + 1], w6[:, DT_T + j : DT_T + j + 1]
                    )

            # masks for all batches + bias
            pm_all = const_pool.tile([128, NB, PT_T], I32, name="pm_all")
            nc.gpsimd.dma_start(pm_all[:], pmask[:].rearrange("n (t p) -> p n t", p=128))
            qm_all = const_pool.tile([128, NB, QT_T], I32, name="qm_all")
            nc.gpsimd.dma_start(qm_all[:], qmask[:].rearrange("n (t q) -> q n t", q=128))
            b_sb = const_pool.tile([128, 1], F32, name="b_sb")
            nc.gpsimd.dma_start(b_sb[:], b_in[0:1].partition_broadcast(128))

            def emit_batch(bi):
                p2q_dst = out_p2q[bi].rearrange("(t p) d -> p t d", p=128)
                co_dst = out_co[bi].rearrange("(t p) d -> p t d", p=128)

                # ---- phase: input DMAs (SP queue) -----------------------
                q_sb = small.tile([128, QT_T, D], F32R, name="q_sb", tag="q_sb")
                q_src = question[bi].rearrange("(t q) d -> q t d", q=128).bitcast(F32R)
                p_sb = big.tile([128, PT_T, D], F32R, name="p_sb", tag="p_sb")
                p_src = passage[bi].rearrange("(t p) d -> p t d", p=128).bitcast(F32R)
                nc.sync.dma_start(q_sb[:, 0:1, :], q_src[:, 0:1, :])
                nc.sync.dma_start(p_sb[:, 0:2, :], p_src[:, 0:2, :])
                nc.sync.dma_start(q_sb[:, 1:2, :], q_src[:, 1:2, :])
                for c in range(1, 4):
                    nc.sync.dma_start(
                        p_sb[:, 2 * c : 2 * c + 2, :], p_src[:, 2 * c : 2 * c + 2, :]
                    )
                yield "dma"

                # ---- phase: head (masks, Q transposes, sq, g, qb) -------
                pm_f = small.tile([128, PT_T], F32, name="pm_f", tag="pm_f")
                nc.vector.tensor_copy(pm_f[:], pm_all[:, bi])
                kp = small.tile([128, PT_T], F32, name="kp", tag="kp")
                nc.vector.tensor_scalar(
                    kp[:], pm_f[:], -1.0, 1.0, mybir.AluOpType.mult, mybir.AluOpType.add
                )
                qm_f = small.tile([128, QT_T], F32, name="qm_f", tag="qm_f")
                nc.vector.tensor_copy(qm_f[:], qm_all[:, bi])
                # qmb = qm*MASK + b
                qmb = small.tile([128, QT_T], F32, name="qmb", tag="qmb")
                nc.vector.tensor_scalar(
                    qmb[:],
                    qm_f[:],
                    MASK,
                    b_sb[:, 0:1],
                    mybir.AluOpType.mult,
                    mybir.AluOpType.add,
                )

                # Q transposes: per-j bank holds both q-tiles side by side
                qt_b = small.tile([128, DT_T, QLEN], BF16, name="qt_b", tag="qt_b")
                qwt = small.tile([128, DT_T, QLEN], BF16, name="qwt", tag="qwt")
                for j in range(DT_T):
                    tq = tp_ps.tile([128, 256], F32R, name=f"tq{j}", tag="tp")
                    for t4 in range(QT_T):
                        nc.tensor.transpose(
                            tq[:, t4 * 128 : (t4 + 1) * 128],
                            q_sb[:, t4, j * 128 : (j + 1) * 128],
                            ident_r,
                        )
                    # QwT = QT * w_pq (per-partition d scale), bf16
                    nc.vector.tensor_scalar_mul(qwt[:, j, :], tq[:], w_pq[:, j : j + 1])
                    nc.scalar.copy(qt_b[:, j, :], tq[:])

                # qb = [Q | 1 | 1] in bf16, [q_part, tq, 258]
                qb = small.tile([128, QT_T, QLEN + 2], BF16, name="qb", tag="qb")
                for t4 in range(QT_T):
                    nc.gpsimd.tensor_copy(qb[:, t4, 0:QLEN], q_sb[:, t4, :].bitcast(F32))
                    nc.gpsimd.tensor_copy(qb[:, t4, QLEN : QLEN + 2], ones_b[:])

                # misc PSUM bank: cols 0:4 = sq (dup pairs), 4:20 = sp
                mi = mi_ps.tile([128, 4 + 2 * PT_T], F32, name="mi", tag="mi")
                sq = mi[:, 0 : 2 * QT_T]
                for tq_i in range(QT_T):
                    for j in range(DT_T):
                        nc.tensor.matmul(
                            sq[:, 2 * tq_i : 2 * tq_i + 2],
                            qt_b[:, j, tq_i * 128 : (tq_i + 1) * 128],
                            w_q_r[:, j, :],
                            start=(j == 0),
                            stop=(j == DT_T - 1),
                        )
                # g = exp(sq + qm*MASK + b) per q-partition, [128, QT_T]
                gb = small.tile([128, QT_T], F32, name="gb", tag="gb")
                for tq_i in range(QT_T):
                    nc.vector.tensor_add(
                        gb[:, tq_i : tq_i + 1],
                        sq[:, 2 * tq_i : 2 * tq_i + 1],
                        qmb[:, tq_i : tq_i + 1],
                    )
                g = small.tile([128, QT_T], F32, name="g", tag="g")
                nc.scalar.activation(g[:], gb[:], AF.Exp)
                yield "head"

                # ---- per-batch big tiles --------------------------------
                pt_b = big.tile([128, DT_T, PLEN], BF16, name="pt_b", tag="pt_b")
                # E0 pairs: [p_part, pair, 512] bf16 (tile 2u in cols 0:256)
                e_sb = big.tile([128, PT_T // 2, 512], BF16, name="e_sb", tag="e_sb")
                et_sb = big.tile([128, QT_T, PLEN], BF16, name="et_sb", tag="et_sb")
                ph = big.tile([128, PT_T, D + 2], BF16, name="ph", tag="ph")
                q2p = small.tile([128, QT_T, D], BF16, name="q2p", tag="q2p")
                p2q_sb = big.tile([128, PT_T, D], F32, name="p2q_sb", tag="p2q_sb")
                co_sb = big.tile([128, PT_T, D], F32, name="co_sb", tag="co_sb")
                rp = small.tile([128, PT_T], F32, name="rp", tag="rp")
                sp = mi[:, 2 * QT_T : 2 * QT_T + 2 * PT_T]

                def emit_ap(t):
                    # p2q attention for tile t (K = q over 2 q-tiles)
                    ap_ = at_ps.tile([128, QLEN + 2], F32, name="ap_", tag="at")
                    for tq_i in range(QT_T):
                        nc.tensor.matmul(
                            ap_[:],
                            et_sb[:, tq_i, t * 128 : (t + 1) * 128],
                            qb[:, tq_i, :],
                            start=(tq_i == 0),
                            stop=(tq_i == QT_T - 1),
                        )
                    v2 = small.tile([128, 1], F32, name="v2", tag="v2")
                    nc.vector.reciprocal(v2[:], ap_[:, QLEN : QLEN + 1])
                    nc.vector.tensor_mul(rp[:, t : t + 1], v2[:], kp[:, t : t + 1])
                    if t % 2 == 0:
                        nc.scalar.activation(
                            p2q_sb[:, t, :], ap_[:, 0:QLEN], AF.Copy,
                            scale=rp[:, t : t + 1],
                        )
                    else:
                        nc.vector.tensor_scalar_mul(
                            p2q_sb[:, t, :], ap_[:, 0:QLEN], rp[:, t : t + 1]
                        )
                    nc.sync.dma_start(p2q_dst[:, t : t + 1, :], p2q_sb[:, t : t + 1, :])

                def emit_co(t):
                    co = at_ps.tile([128, D], F32, name="co", tag="at")
                    for tq_i in range(QT_T):
                        nc.tensor.matmul(
                            co[:],
                            et_sb[:, tq_i, t * 128 : (t + 1) * 128],
                            q2p[:, tq_i, :],
                            start=(tq_i == 0),
                            stop=(tq_i == QT_T - 1),
                        )
                    if t % 2 == 1:
                        nc.scalar.activation(
                            co_sb[:, t, :], co[:], AF.Copy, scale=rp[:, t : t + 1]
                        )
                    else:
                        nc.vector.tensor_scalar_mul(
                            co_sb[:, t, :], co[:], rp[:, t : t + 1]
                        )
                    nc.gpsimd.dma_start(co_dst[:, t : t + 1, :], co_sb[:, t : t + 1, :])

                def emit_grp_scores(grp):
                    t_lo = grp * 4
                    # PT transposes for this half (per-j bank of 4 tiles)
                    for j in range(DT_T):
                        tp = tp_ps.tile([128, 512], F32R, name="tp", tag="tp")
                        for t4 in range(4):
                            t = t_lo + t4
                            nc.tensor.transpose(
                                tp[:, t4 * 128 : (t4 + 1) * 128],
                                p_sb[:, t, j * 128 : (j + 1) * 128],
                                ident_r,
                            )
                        if j == 0:
                            nc.vector.tensor_copy(
                                pt_b[:, j, grp * 512 : (grp + 1) * 512], tp[:]
                            )
                        else:
                            nc.scalar.copy(
                                pt_b[:, j, grp * 512 : (grp + 1) * 512], tp[:]
                            )
                    # S0 pairs -> paired exp -> bf16 E0; sp rides along
                    for pair in range(2):
                        t0 = t_lo + 2 * pair
                        s0b = s0_ps.tile([128, 512], F32, name="s0b", tag="s0")
                        for half in range(2):
                            t = t0 + half
                            for j in range(DT_T):
                                nc.tensor.matmul(
                                    s0b[:, half * 256 : (half + 1) * 256],
                                    pt_b[:, j, t * 128 : (t + 1) * 128],
                                    qwt[:, j, :],
                                    start=(j == 0),
                                    stop=(j == DT_T - 1),
                                )
                            for j in range(DT_T):
                                nc.tensor.matmul(
                                    sp[:, 2 * t : 2 * t + 2],
                                    pt_b[:, j, t * 128 : (t + 1) * 128],
                                    w_p_r[:, j, :],
                                    start=(j == 0),
                                    stop=(j == DT_T - 1),
                                )
                        nc.scalar.activation(e_sb[:, t0 // 2, :], s0b[:], AF.Exp)
                    # ET' = E0^T * g via PE transposes of E0 chunks
                    for tq_i in range(QT_T):
                        etp = et_ps.tile([128, 512], BF16, name="etp", tag="et")
                        for t4 in range(4):
                            t = t_lo + t4
                            nc.tensor.transpose(
                                etp[:, t4 * 128 : (t4 + 1) * 128],
                                e_sb[:, t // 2, (t % 2) * 256 + tq_i * 128 :
                                     (t % 2) * 256 + (tq_i + 1) * 128],
                                ident_b,
                            )
                        if tq_i == 0:
                            nc.vector.tensor_scalar_mul(
                                et_sb[:, tq_i, grp * 512 : (grp + 1) * 512],
                                etp[:],
                                g[:, tq_i : tq_i + 1],
                            )
                        else:
                            nc.scalar.activation(
                                et_sb[:, tq_i, grp * 512 : (grp + 1) * 512],
                                etp[:],
                                AF.Copy,
                                scale=g[:, tq_i : tq_i + 1],
                            )

                # ---- phase g0: first half scores + its p2q --------------
                emit_grp_scores(0)
                for t in range(0, 4):
                    emit_ap(t)
                yield "g0"

                # ---- phase g1a: second half scores + h/ph ---------------
                emit_grp_scores(1)
                # h = exp(sp) * kp ; ph = [P*h | h | h] bf16
                h_raw = small.tile([128, 2 * PT_T], F32, name="h_raw", tag="h_raw")
                nc.scalar.activation(h_raw[:], sp[:], AF.Exp)
                hk = small.tile([128, 2 * PT_T], F32, name="hk", tag="hk")
                for t in range(PT_T):
                    nc.gpsimd.tensor_mul(
                        hk[:, 2 * t : 2 * t + 1],
                        h_raw[:, 2 * t : 2 * t + 1],
                        kp[:, t : t + 1],
                    )
                for t in range(PT_T):
                    nc.gpsimd.tensor_scalar_mul(
                        ph[:, t, 0:D],
                        p_sb[:, t, :].bitcast(F32),
                        hk[:, 2 * t : 2 * t + 1],
                    )
                    nc.gpsimd.tensor_copy(ph[:, t, D : D + 1], hk[:, 2 * t : 2 * t + 1])
                    nc.gpsimd.tensor_copy(
                        ph[:, t, D + 1 : D + 2], hk[:, 2 * t : 2 * t + 1]
                    )
                yield "g1a"

                # ---- phase g1b: q2p, remaining p2q, coattention ---------
                for tq_i in range(QT_T):
                    aq = at_ps.tile([128, D + 2], F32, name="aq", tag="at")
                    for t in range(PT_T):
                        nc.tensor.matmul(
                            aq[:],
                            e_sb[:, t // 2, (t % 2) * 256 + tq_i * 128 :
                                 (t % 2) * 256 + (tq_i + 1) * 128],
                            ph[:, t, :],
                            start=(t == 0),
                            stop=(t == PT_T - 1),
                        )
                    u2 = small.tile([128, 1], F32, name="u2", tag="u2")
                    nc.vector.reciprocal(u2[:], aq[:, D : D + 1])
                    nc.vector.tensor_scalar_mul(q2p[:, tq_i, :], aq[:, 0:D], u2[:])
                for t in range(4, PT_T):
                    emit_ap(t)
                    emit_co(t - 4)
                for t in range(4, PT_T):
                    emit_co(t)
                yield "g1b"

            # interleaved emission: batch 1's input DMAs and head overlap
            # batch 0's compute; see docstring.
            gens = [emit_batch(bi) for bi in range(NB)]
            if NB == 2:
                g0, g1 = gens
                next(g0)  # b0 dma
                next(g0)  # b0 head
                next(g0)  # b0 g0
                next(g1)  # b1 dma
                next(g0)  # b0 g1a
                next(g1)  # b1 head
                next(g0)  # b0 g1b
                next(g1)  # b1 g0
                next(g1)  # b1 g1a
                next(g1)  # b1 g1b
                for g in gens:
                    for _ in g:
                        pass
            else:
                for g in gens:
                    for _ in g:
                        pass

    return nc


_nc_cache = None


def kernel(passage, question, passage_mask, question_mask, W, b):
    global _nc_cache
    _install_bir_wait_split()
    if _nc_cache is None:
        _nc_cache = build_nc()
    nc = _nc_cache

    passage = np.ascontiguousarray(passage, dtype=np.float32)
    question = np.ascontiguousarray(question, dtype=np.float32)
    passage_mask = np.ascontiguousarray(passage_mask, dtype=np.int32)
    question_mask = np.ascontiguousarray(question_mask, dtype=np.int32)
    W = np.ascontiguousarray(W, dtype=np.float32)
    b = np.ascontiguousarray(b, dtype=np.float32)

    in_maps = []
    for c in range(N_CORES):
        s = slice(c * NB, (c + 1) * NB)
        in_maps.append(
            {
                "passage": passage[s],
                "question": question[s],
                "passage_mask": passage_mask[s],
                "question_mask": question_mask[s],
                "W": W,
                "b": b,
            }
        )
    res = run_bass_kernel_spmd(nc, in_maps, list(range(N_CORES)))
    p2q = np.concatenate([r["p2q"] for r in res.results], axis=0)
    coatt = np.concatenate([r["coatt"] for r in res.results], axis=0)
    return p2q, coatt
